# revision 3
# baseline (speedup 1.0000x reference)
"""DRCLoss kernel v3 for 8 Trainium2 NeuronCores (Bass/Tile, SPMD).

Math: loss = mean_i[ relu(l1_i + l2_i + d12_i - neg_i + 0.1) + max(l1_i, l2_i) ]
  where neg_i = min over non-self columns of cdist(ts, [ts; im1; im2])[i, :].

v3 strategy (column-sharded: each core owns 1536 of the 12288 columns j,
all 4096 rows i):
  - Device computes s[j, i] = 2*r_j.x_i - rsq[j] with output partitions = j.
    Only the masked column-min survives on device; l1/l2/d12/xsq are exact
    host math, so the output is just the running row-max of s.
  - Stationary (R-chunk) is reused across all 8 i-blocks of a j-tile; PSUM
    is used as two 4-bank [128, 2048] quads, so one fused op drains 4 tiles.
  - Per quad, one of two fold paths:
      D: DVE scalar_tensor_tensor  acc = max(acc, psum + (-rsq_j))  (fp32)
      A: ScalarE activation copy psum+bias -> fp16, DVE tensor_max fold
    balancing the drain work across ScalarE and DVE.
  - Self-column exclusion: each j-tile has one 128-long self diagonal. A
    per-core rotation of the i axis (shift by 1536c) makes the diagonal
    position uniform across cores, so a single eye x shifted-identity bf16
    matmul per j-tile adds -30000 in PSUM (SPMD-safe, no vector-engine cost).
  - Host finishes: negsq = xsq - max over cores/partitions, sqrt/relu/mean
    in float64.
"""

import sys

if "/opt/trn_rl_repo" not in sys.path:
    sys.path.insert(0, "/opt/trn_rl_repo")

from contextlib import ExitStack

import ml_dtypes
import numpy as np

import concourse.bass as bass
import concourse.tile as tile
from concourse import mybir
from concourse.bass_utils import run_bass_kernel_spmd

BF16 = ml_dtypes.bfloat16
F8 = ml_dtypes.float8_e4m3

B = 4096          # rows
D = 512           # feature dim
M = 8             # cores
JC = 3 * B // M   # columns per core (1536)
NJT = JC // 128   # j-tiles per core (12)
IH = B // 2       # rows per half (2048)

# quads folded via the D path (DVE stt direct from PSUM); rest via A path
# (ScalarE bias-copy + DVE fp16 fold). 6 of 24 per the engine balance.
D_QUADS = {(1, 3), (1, 7), (1, 11), (0, 3), (0, 7), (0, 11)}

LAST_RESULTS = None

_NC_CACHE = None


def _install_ntff_hook():
    """Provide antenv.axon_hooks (missing in this image) so trace=True can
    capture NTFF profiles through libaxon_pjrt.so."""
    try:
        import antenv.axon_hooks  # noqa: F401

        return
    except ImportError:
        pass
    try:
        import types

        import antenv
        from trn_agent_boot.trn_boot import _ntff_profile_via_ctypes

        mod = types.ModuleType("antenv.axon_hooks")
        mod._hook = None

        def set_axon_ntff_profile_hook(h):
            mod._hook = h

        def get_axon_ntff_profile_hook():
            return mod._hook

        mod.set_axon_ntff_profile_hook = set_axon_ntff_profile_hook
        mod.get_axon_ntff_profile_hook = get_axon_ntff_profile_hook
        sys.modules["antenv.axon_hooks"] = mod
        antenv.axon_hooks = mod
        hook = _ntff_profile_via_ctypes("/opt/axon/libaxon_pjrt.so")
        if hook is not None:
            mod._hook = hook
    except Exception:
        pass


def _split_multi_waits(nc):
    """This walrus build allows only ONE embedded sync wait per instruction.
    Hoist extra waits onto standalone EventSemaphore instructions inserted
    just before the owner (same engine, so program order is preserved)."""
    import bass_rust

    ctr = 0
    for blk in nc.m.functions[0].blocks:
        il = blk.instructions
        new = []
        for inst in il:
            si = getattr(inst, "sync_info", None)
            waits = list(si.on_wait) if si is not None else []
            if len(waits) > 1:
                for w in waits[:-1]:
                    ev = bass_rust.InstEventSemaphore(name=f"wsplit_{ctr}")
                    ctr += 1
                    ev.engine = inst.engine
                    ev.sync_info = bass_rust.SyncInfo(on_wait=[w], on_update=[])
                    new.append(ev)
                inst.sync_info = bass_rust.SyncInfo(
                    on_wait=[waits[-1]], on_update=list(si.on_update)
                )
            new.append(inst)
        il[:] = new


def _build_nc():
    nc = bass.Bass()
    f32 = mybir.dt.float32
    f16 = mybir.dt.float16
    bf16 = mybir.dt.bfloat16
    f8 = mybir.dt.float8e4
    DR = mybir.MatmulPerfMode.DoubleRow
    add = mybir.AluOpType.add
    mx = mybir.AluOpType.max

    # stationary R-chunks [pk, jt, kh, dr, j]
    st_d = nc.dram_tensor("st8", [128, NJT, 2, 2, 128], f8, kind="ExternalInput")
    # moving X^T, i-rotated per core, ihalf-major [pk, ih, kh, dr, i]
    xt_d = nc.dram_tensor("xt8", [128, 2, 2, 2, IH], f8, kind="ExternalInput")
    rsqT_d = nc.dram_tensor("rsqT", [128, NJT], f32, kind="ExternalInput")
    mask_d = nc.dram_tensor("mask", [128, 4, 2, 512], f8, kind="ExternalInput")
    eye_d = nc.dram_tensor("eye", [128, 2, 128], f8, kind="ExternalInput")
    oD_d = nc.dram_tensor("oD", [128, B], f32, kind="ExternalOutput")
    oA_d = nc.dram_tensor("oA", [128, B], f16, kind="ExternalOutput")

    with ExitStack() as ctx:
        tc = ctx.enter_context(tile.TileContext(nc))
        const = ctx.enter_context(tc.tile_pool(name="const", bufs=1))
        hpp = ctx.enter_context(tc.tile_pool(name="hp", bufs=3))
        psump = ctx.enter_context(tc.tile_pool(name="psum", bufs=2, space="PSUM"))

        def dummy_mm(lhs_ap, rhs_ap):
            pw = psump.tile([128, 2048], f32, tag="q", name="pdum")
            nc.tensor.matmul(pw[: lhs_ap.shape[-1], : rhs_ap.shape[-1]],
                             lhs_ap, rhs_ap, start=True, stop=True)

        # ihalf=1 moving slice arrives first (processed first, mask-free)
        xt1 = const.tile([128, 2, 2, IH], f8, tag="xt1")
        nc.sync.dma_start(out=xt1, in_=xt_d[:, 1])
        dummy_mm(xt1[:, 0, 0, 0:4], xt1[:, 0, 0, 0:8])
        # short warmup burst to shake the PE HAM throttle while DMAs land
        for _ in range(8):
            pw = psump.tile([128, 2048], f32, tag="q", name="pwarm")
            nc.tensor.matmul(pw[:, 0:512], xt1[:, 0, 0, 0:128], xt1[:, 0, 0, 0:512],
                             start=True, stop=True)

        st = const.tile([128, NJT, 2, 2, 128], f8, tag="st")
        nc.sync.dma_start(out=st, in_=st_d[:, :])
        dummy_mm(st[:, 0, 0, 0, 0:4], st[:, 0, 0, 0, 0:8])

        rsqT = const.tile([128, NJT], f32, tag="rsqT")
        nc.sync.dma_start(out=rsqT, in_=rsqT_d[:, :])
        vabs = const.tile([128, 1], f32, tag="vabs")
        nc.vector.tensor_copy(vabs, rsqT[:, 0:1])
        sabs = const.tile([128, 1], f32, tag="sabs")
        nc.scalar.copy(sabs, rsqT[:, 0:1])

        xt0 = const.tile([128, 2, 2, IH], f8, tag="xt0")
        nc.sync.dma_start(out=xt0, in_=xt_d[:, 0])
        dummy_mm(xt0[:, 0, 0, 0:4], xt0[:, 0, 0, 0:8])

        maskr = const.tile([128, 4, 2, 512], f8, tag="maskr")
        nc.sync.dma_start(out=maskr, in_=mask_d[:, :])
        dummy_mm(maskr[:, 0, 0, 0:4], maskr[:, 0, 0, 0:8])
        eye = const.tile([128, 2, 128], f8, tag="eye")
        nc.sync.dma_start(out=eye, in_=eye_d[:, :])
        dummy_mm(eye[:, 0, 0:4], eye[:, 0, 0:8])

        accD = const.tile([128, B], f32, tag="accD")
        nc.gpsimd.memset(accD, -60000.0)
        accA = const.tile([128, B], f16, tag="accA")
        nc.vector.memset(accA, -60000.0)

        for ihalf in (1, 0):
            xt = xt1 if ihalf == 1 else xt0
            has_mask = ihalf == 0
            dsl = accD[:, ihalf * IH : (ihalf + 1) * IH]
            asl = accA[:, ihalf * IH : (ihalf + 1) * IH]
            for jt in range(NJT):
                q = psump.tile([128, 2048], f32, tag="q", name="q")
                mib = jt // 4 if has_mask else -1
                for kh in range(2):
                    for ib in range(4):
                        nc.tensor.matmul(
                            q[:, ib * 512 : (ib + 1) * 512],
                            st[:, jt, kh],
                            xt[:, kh, :, ib * 512 : (ib + 1) * 512],
                            start=(kh == 0),
                            stop=(kh == 1 and ib != mib),
                            perf_mode=DR,
                        )
                if has_mask:
                    nc.tensor.matmul(
                        q[:, mib * 512 : (mib + 1) * 512],
                        eye,
                        maskr[:, jt % 4],
                        start=False,
                        stop=True,
                        perf_mode=DR,
                    )
                if (ihalf, jt) in D_QUADS:
                    nc.vector.scalar_tensor_tensor(
                        out=dsl, in0=q, scalar=rsqT[:, jt : jt + 1], in1=dsl,
                        op0=add, op1=mx,
                    )
                else:
                    hp = hpp.tile([128, 2048], f16, tag="hp")
                    nc.scalar.add(hp, q, rsqT[:, jt : jt + 1])
                    nc.vector.tensor_max(asl, asl, hp)
            # stream this half's results out while the other half computes
            nc.gpsimd.dma_start(out=oD_d[:, ihalf * IH : (ihalf + 1) * IH], in_=dsl)
            nc.sync.dma_start(out=oA_d[:, ihalf * IH : (ihalf + 1) * IH], in_=asl)

    _split_multi_waits(nc)
    return nc


def _host_inputs(feature_ts, feature_image1, feature_image2):
    ts = np.ascontiguousarray(feature_ts, dtype=np.float32)
    im1 = np.ascontiguousarray(feature_image1, dtype=np.float32)
    im2 = np.ascontiguousarray(feature_image2, dtype=np.float32)

    R = np.concatenate([ts, im1, im2], 0)                      # [3B, D]
    rsq = (R.astype(np.float64) ** 2).sum(1)                   # [3B]
    x8 = ts.astype(F8)                                         # [B, D]
    r2_8 = (2.0 * R.astype(F8).astype(np.float32)).astype(F8)  # exact doubling

    maskr = np.zeros((128, 4, 2, 512), dtype=F8)
    p = np.arange(64)
    for toff in range(4):
        for dr in range(2):
            maskr[p, toff, dr, 128 * toff + p + 64 * dr] = F8(-448.0)
    eye = np.zeros((128, 2, 128), dtype=F8)
    for dr in range(2):
        eye[p, dr, p + 64 * dr] = F8(64.0)

    in_maps = []
    for c in range(M):
        jsl = slice(c * JC, (c + 1) * JC)
        # st8[pk, jt, kh, dr, j] = r2_8[c*JC + 128*jt + j, kh*256 + dr*128 + pk]
        st8 = np.ascontiguousarray(
            r2_8[jsl].reshape(NJT, 128, 2, 2, 128).transpose(4, 0, 2, 3, 1)
        )
        # i-rotation: logical i -> physical (i + c*JC) % B
        iperm = (np.arange(B) + c * JC) % B
        xr = x8[iperm]                                         # [B, D] rotated
        # xt8[pk, ih, kh, dr, i] = xr[ih*IH + i, kh*256 + dr*128 + pk]
        xt8 = np.ascontiguousarray(
            xr.reshape(2, IH, 2, 2, 128).transpose(4, 0, 2, 3, 1)
        )
        rsqT = np.ascontiguousarray(
            (-rsq[jsl].reshape(NJT, 128).T).astype(np.float32)
        )
        in_maps.append(
            {"st8": st8, "xt8": xt8, "rsqT": rsqT, "mask": maskr, "eye": eye}
        )
    return in_maps


def _combine(outs, feature_ts, feature_image1, feature_image2):
    ts = np.asarray(feature_ts, dtype=np.float64)
    im1 = np.asarray(feature_image1, dtype=np.float64)
    im2 = np.asarray(feature_image2, dtype=np.float64)
    l1 = np.sqrt(((ts - im1) ** 2).sum(1))
    l2 = np.sqrt(((ts - im2) ** 2).sum(1))
    d12 = np.sqrt(((im1 - im2 + 1e-6) ** 2).sum(1))
    xsq = (ts ** 2).sum(1)

    smax = np.full(B, -np.inf)
    for c, (oD, oA) in enumerate(outs):
        s = np.maximum(
            np.asarray(oD, dtype=np.float64).max(axis=0),
            np.asarray(oA, dtype=np.float64).max(axis=0),
        )                                                       # [B] logical i
        phys = (np.arange(B) + c * JC) % B
        np.maximum.at(smax, phys, s)
    negsq = xsq - smax
    neg = np.sqrt(np.maximum(negsq, 0.0))
    trip = np.maximum(l1 + l2 + d12 - neg + 0.1, 0.0) + np.maximum(l1, l2)
    return np.float32(trip.sum() / B)


def kernel(feature_ts, feature_image1, feature_image2, _trace=False):
    global _NC_CACHE, LAST_RESULTS
    if _NC_CACHE is None:
        _NC_CACHE = _build_nc()
    if _trace:
        _install_ntff_hook()
    in_maps = _host_inputs(feature_ts, feature_image1, feature_image2)
    res = run_bass_kernel_spmd(_NC_CACHE, in_maps, list(range(M)), trace=_trace)
    LAST_RESULTS = res
    return _combine(
        [(res.results[c]["oD"], res.results[c]["oA"]) for c in range(M)],
        feature_ts, feature_image1, feature_image2,
    )
